# revision 13
# baseline (speedup 1.0000x reference)
"""DetectionTargetLayer (Mask R-CNN target sampling) on 8 Trainium2 cores.

Self-contained: builds a Bass/Tile SPMD program, runs it on cores 0-7 via
run_bass_kernel_spmd, reassembles full outputs.

Sharding: core r handles (image b = r // 4, ROI-shard rr = r % 4).
Each core redundantly computes the (tiny) selection pipeline for its image,
then computes mask crops for its 17 ROIs by sweeping the image's gt_masks in
4 y-slabs of 128 rows.
"""
import sys

if "/opt/trn_rl_repo" not in sys.path:
    sys.path.insert(0, "/opt/trn_rl_repo")

import numpy as np

import concourse.bass as bass
import concourse.mybir as mybir
import concourse.tile as tile
from concourse import bacc
from concourse.bass_utils import run_bass_kernel_spmd

F32 = mybir.dt.float32
F16 = mybir.dt.float16
BF16 = mybir.dt.bfloat16
I32 = mybir.dt.int32
U8 = mybir.dt.uint8
AL = mybir.AluOpType
AX = mybir.AxisListType
ds = bass.ds

B = 2
N = 2000          # proposals per image
G = 100           # gt slots per image
T = 200           # output rois per image
PM = 66           # POS_MAX
S = 28            # mask size
HH = 512
NP = 125          # n = p*16 + i layout
NI = 16
NJ = 17           # ROIs per core
JLO = [0, 17, 34, 49]
R_RATIO = np.float32(1.0 / 0.33)
BIG = 1.0e8


def build_program():
    nc = bacc.Bacc("TRN2", target_bir_lowering=False, debug=False, num_devices=8)

    t_prop = nc.dram_tensor("prop", [N, 4], F32, kind="ExternalInput")
    t_aux = nc.dram_tensor("aux", [N, 4], F32, kind="ExternalInput")   # ay,by,ax,bx
    t_gtb = nc.dram_tensor("gtb", [G, 4], F32, kind="ExternalInput")
    t_gtc = nc.dram_tensor("gtc", [G, 1], I32, kind="ExternalInput")
    t_msk = nc.dram_tensor("mskin", [HH, HH * G], U8, kind="ExternalInput")
    t_meta = nc.dram_tensor("meta", [1, 8], I32, kind="ExternalInput")  # [jlo]

    o_rois = nc.dram_tensor("o_rois", [T, 4], F32, kind="ExternalOutput")
    o_cls = nc.dram_tensor("o_cls", [T, 1], I32, kind="ExternalOutput")
    o_dlt = nc.dram_tensor("o_dlt", [T, 4], F32, kind="ExternalOutput")
    # stored [s, j, t]; host transposes to [j, s, t]
    o_msk = nc.dram_tensor("o_msk", [S, T, S], F32, kind="ExternalOutput")

    # internal DRAM scratch
    d_int = nc.dram_tensor("d_int", [N, G], F32)
    d_uni = nc.dram_tensor("d_uni", [N, G], F32)
    d_pack = nc.dram_tensor("d_pack", [8, G], F32)
    d_scal = nc.dram_tensor("d_scal", [1, 16], F32)
    d_jc = nc.dram_tensor("d_jc", [1, T], F32)
    d_y01 = nc.dram_tensor("d_y01", [PM, 64], F16)
    d_x01 = nc.dram_tensor("d_x01", [PM, 64], F16)
    d_wyT = nc.dram_tensor("d_wyT", [S, PM], F32)
    d_wx = nc.dram_tensor("d_wx", [PM, S], F32)
    d_asg = nc.dram_tensor("d_asg", [1, G], F32)

    with tile.TileContext(nc) as tc:
        _body(nc, tc, locals())
    nc.compile()
    return nc


def _body(nc, tc, tn):
    t_prop, t_aux, t_gtb, t_gtc, t_msk, t_meta = (
        tn["t_prop"], tn["t_aux"], tn["t_gtb"], tn["t_gtc"], tn["t_msk"], tn["t_meta"])
    o_rois, o_cls, o_dlt, o_msk = tn["o_rois"], tn["o_cls"], tn["o_dlt"], tn["o_msk"]
    d_int, d_uni, d_pack, d_scal, d_jc = (
        tn["d_int"], tn["d_uni"], tn["d_pack"], tn["d_scal"], tn["d_jc"])
    d_y01, d_x01, d_wyT, d_wx, d_asg = (
        tn["d_y01"], tn["d_x01"], tn["d_wyT"], tn["d_wx"], tn["d_asg"])

    import contextlib
    stack = contextlib.ExitStack()
    glob = stack.enter_context(tc.tile_pool(name="glob", bufs=1))

    # ---------- constants ----------
    # n-iota: n = p*16 + i
    nio_i = glob.tile([NP, NI], I32, name="nio_i")
    nc.gpsimd.iota(nio_i[:], pattern=[[1, NI]], base=0, channel_multiplier=NI)
    nio_f = glob.tile([NP, NI], F32, name="nio_f")
    nc.vector.tensor_copy(nio_f[:], nio_i[:])
    # p-iota columns [p, p+100]
    pio_i = glob.tile([128, 2], I32, name="pio_i")
    nc.gpsimd.iota(pio_i[:], pattern=[[100, 2]], base=0, channel_multiplier=1)
    pio_f = glob.tile([128, 2], F32, name="pio_f")
    nc.vector.tensor_copy(pio_f[:], pio_i[:])
    # g-iota per partition (0..99 on partition dim)
    gio_f = pio_f[0:G, 0:1]
    # j-iota row on partition 0 [1, 200]
    jio_i = glob.tile([1, T], I32, name="jio_i")
    nc.gpsimd.iota(jio_i[:], pattern=[[1, T]], base=0, channel_multiplier=0)
    jio_f = glob.tile([1, T], F32, name="jio_f")
    nc.vector.tensor_copy(jio_f[:], jio_i[:])
    # free-iota row replicated on all partitions [128, 200]
    jio_all_i = glob.tile([128, T], I32, name="jio_all_i")
    nc.gpsimd.iota(jio_all_i[:], pattern=[[1, T]], base=0, channel_multiplier=0)
    jio_all_f = glob.tile([128, T], F32, name="jio_all_f")
    nc.vector.tensor_copy(jio_all_f[:], jio_all_i[:])
    # f-iota row [128, 128] (same on each partition) for tril compare
    fio_i = glob.tile([128, 128], I32, name="fio_i")
    nc.gpsimd.iota(fio_i[:], pattern=[[1, 128]], base=0, channel_multiplier=0)
    fio_f = glob.tile([128, 128], F32, name="fio_f")
    nc.vector.tensor_copy(fio_f[:], fio_i[:])
    # s-iota [G, 28] (0..27 per partition)
    sio_i = glob.tile([G, S], I32, name="sio_i")
    nc.gpsimd.iota(sio_i[:], pattern=[[1, S]], base=0, channel_multiplier=0)
    sio_f = glob.tile([G, S], F32, name="sio_f")
    nc.vector.tensor_copy(sio_f[:], sio_i[:])
    # y/x-chunk iota [128, 4]: p + 128*c, fp16
    cio_i = glob.tile([128, 4], I32, name="cio_i")
    nc.gpsimd.iota(cio_i[:], pattern=[[128, 4]], base=0, channel_multiplier=1)
    cio_16 = glob.tile([128, 4], F16, name="cio_16")
    nc.vector.tensor_copy(cio_16[:], cio_i[:])
    # tril (k <= m) [128, 128] f32
    tril = glob.tile([128, 128], F32, name="tril")
    nc.vector.tensor_tensor(out=tril[:], in0=pio_f[:, 0:1].to_broadcast([128, 128]),
                            in1=fio_f[:], op=AL.is_le)
    ones_col = glob.tile([128, 1], F32, name="ones_col")
    nc.vector.memset(ones_col[:], 1.0)

    # ---------- load small inputs ----------
    prop = glob.tile([NP, NI, 4], F32, name="prop")
    nc.sync.dma_start(prop[:], t_prop[:].rearrange("(p i) c -> p i c", i=NI))
    gtb = glob.tile([G, 4], F32, name="gtb")
    nc.sync.dma_start(gtb[:], t_gtb[:])
    gtc = glob.tile([G, 1], I32, name="gtc")
    nc.sync.dma_start(gtc[:], t_gtc[:])
    meta = glob.tile([1, 8], I32, name="meta")
    nc.sync.dma_start(meta[:], t_meta[:])

    # ---------- gt pack [G, 8] -> DRAM -> bcast [128, 8, G] ----------
    # cols: 0..3 = gt y1,x1,y2,x2 ; 4 = area2 ; 5 = fg ; 6 = fgoff ; 7 = croff
    pack = glob.tile([G, 8], F32, name="pack")
    nc.vector.tensor_copy(pack[:, 0:4], gtb[:])
    gv = glob.tile([G, 1], F32, name="gv")
    nc.vector.tensor_reduce(out=gv[:], in_=gtb[:], axis=AX.X, op=AL.add,
                            apply_absolute_value=True)
    nc.vector.tensor_scalar(out=gv[:], in0=gv[:], scalar1=0.0, scalar2=None,
                            op0=AL.is_gt)
    gh_ = glob.tile([G, 1], F32, name="gh_")
    nc.vector.tensor_tensor(out=gh_[:], in0=gtb[:, 2:3], in1=gtb[:, 0:1], op=AL.subtract)
    gw_ = glob.tile([G, 1], F32, name="gw_")
    nc.vector.tensor_tensor(out=gw_[:], in0=gtb[:, 3:4], in1=gtb[:, 1:2], op=AL.subtract)
    nc.vector.tensor_tensor(out=pack[:, 4:5], in0=gh_[:], in1=gw_[:], op=AL.mult)
    clsf = glob.tile([G, 1], F32, name="clsf")
    nc.vector.tensor_copy(clsf[:], gtc[:])
    fg = glob.tile([G, 1], F32, name="fg")
    nc.vector.tensor_scalar(out=fg[:], in0=clsf[:], scalar1=0.0, scalar2=None, op0=AL.is_gt)
    nc.vector.tensor_tensor(out=pack[:, 5:6], in0=fg[:], in1=gv[:], op=AL.mult)
    cr = glob.tile([G, 1], F32, name="cr")
    nc.vector.tensor_scalar(out=cr[:], in0=clsf[:], scalar1=0.0, scalar2=None, op0=AL.is_lt)
    nc.vector.tensor_tensor(out=cr[:], in0=cr[:], in1=gv[:], op=AL.mult)
    # fgoff = (fg_valid - 1) * BIG ; croff = (cr_valid - 1) * BIG
    nc.vector.tensor_scalar(out=pack[:, 6:7], in0=pack[:, 5:6], scalar1=1.0,
                            scalar2=BIG, op0=AL.subtract, op1=AL.mult)
    nc.vector.tensor_scalar(out=pack[:, 7:8], in0=cr[:], scalar1=1.0,
                            scalar2=BIG, op0=AL.subtract, op1=AL.mult)
    nc.sync.dma_start(d_pack[:].rearrange("q g -> g q"), pack[:])
    packb = glob.tile([128, 8, G], F32, name="packb")
    nc.sync.dma_start(packb[:].rearrange("p q g -> p (q g)"),
                      bass.AP(tensor=d_pack[:].tensor, offset=0,
                              ap=[[0, 128], [1, 8 * G]]))

    def prow(c, psz=NP, isz=NI):
        # pack column c as [psz, isz(bcast), G]
        a = packb[0:psz, c, :]
        return bass.AP(tensor=a.tensor, offset=a.offset,
                       ap=[a.ap[0], [0, isz], a.ap[1]])

    # ---------- phase 0b: iou stats [NP, NI, G] split DVE/GPS over i ----------
    stack0 = contextlib.ExitStack()
    ph0 = stack0.enter_context(tc.tile_pool(name="ph0", bufs=1))
    NSPL = 11  # i-slices 0..10 on DVE, 11..15 on gpsimd

    def big(name):
        return ph0.tile([NP, NI, G], F32, name=name)

    def esl(e, ap):
        if len(ap.shape) != 3:
            return ap
        return ap[:, 0:NSPL, :] if e == 0 else ap[:, NSPL:NI, :]

    engs = [nc.vector, nc.vector]

    def tt2(out, a, b, op):
        nc.vector.tensor_tensor(out=out, in0=a, in1=b, op=op)

    def ts2(out, a, s1, o1, s2=None, o2=None):
        kw = {}
        if o2 is not None:
            kw["op1"] = o2
        nc.vector.tensor_scalar(out=out, in0=a, scalar1=s1, scalar2=s2, op0=o1, **kw)

    def pcol(c):
        # proposal coord c broadcast over g: [NP, NI, G]
        a = prop[:, :, c]
        return bass.AP(tensor=a.tensor, offset=a.offset,
                       ap=[a.ap[0], a.ap[1], [0, G]])

    y1t = big("y1t"); tt2(y1t[:], pcol(0), prow(0), AL.max)
    x1t = big("x1t"); tt2(x1t[:], pcol(1), prow(1), AL.max)
    y2t = big("y2t"); tt2(y2t[:], pcol(2), prow(2), AL.min)
    x2t = big("x2t"); tt2(x2t[:], pcol(3), prow(3), AL.min)
    tt2(x2t[:], x2t[:], x1t[:], AL.subtract)          # dw
    tt2(y2t[:], y2t[:], y1t[:], AL.subtract)          # dh
    ts2(x2t[:], x2t[:], 0.0, AL.max)
    ts2(y2t[:], y2t[:], 0.0, AL.max)
    inter = big("inter"); tt2(inter[:], x2t[:], y2t[:], AL.mult)
    # a1 [NP, NI]
    a1 = ph0.tile([NP, NI], F32, name="a1")
    h1 = ph0.tile([NP, NI], F32, name="h1")
    w1 = ph0.tile([NP, NI], F32, name="w1")
    nc.vector.tensor_tensor(out=h1[:], in0=prop[:, :, 2], in1=prop[:, :, 0], op=AL.subtract)
    nc.vector.tensor_tensor(out=w1[:], in0=prop[:, :, 3], in1=prop[:, :, 1], op=AL.subtract)
    nc.vector.tensor_tensor(out=a1[:], in0=h1[:], in1=w1[:], op=AL.mult)

    def a1b():
        a = a1[:]
        return bass.AP(tensor=a.tensor, offset=a.offset, ap=[a.ap[0], a.ap[1], [0, G]])

    uni = big("uni")
    tt2(uni[:], a1b(), prow(4), AL.add)
    tt2(uni[:], uni[:], inter[:], AL.subtract)
    # spill inter/union
    nc.sync.dma_start(d_int[:].rearrange("(p i) g -> p i g", i=NI), inter[:])
    nc.sync.dma_start(d_uni[:].rearrange("(p i) g -> p i g", i=NI), uni[:])
    # m2 = 2*inter - union ; fg-masked; crowd: c2 = inter - 1e-3*union, crowd-masked
    m2 = big("m2")
    ts2(m2[:], inter[:], 2.0, AL.mult)
    tt2(m2[:], m2[:], uni[:], AL.subtract)
    tt2(m2[:], m2[:], prow(5), AL.mult)
    tt2(m2[:], m2[:], prow(6), AL.add)
    c2 = big("c2")
    ts2(c2[:], uni[:], 1.0e-3, AL.mult)
    tt2(c2[:], inter[:], c2[:], AL.subtract)
    crm = big("crm")

    def crrow(c, psz=NP, isz=NI):
        a = packb[0:psz, c, :]
        return bass.AP(tensor=a.tensor, offset=a.offset,
                       ap=[a.ap[0], [0, isz], a.ap[1]])

    # crowd multiplicative mask: cr_valid = pack col? col7 is croff; need cr mask itself.
    # cr (valid crowd indicator) wasn't packed; derive: crmask = (croff == 0) -> 1 where crowd
    tt2(crm[:], prow(7), prow(7), AL.is_equal)  # placeholder; replaced below
    # Instead compute c2f = c2 * crmask + croff with crmask = (croff >= 0)
    ts2(crm[:], prow(7), -0.5, AL.is_ge)
    tt2(c2[:], c2[:], crm[:], AL.mult)
    tt2(c2[:], c2[:], prow(7), AL.add)

    posmax = ph0.tile([NP, NI], F32, name="posmax")
    crmax = ph0.tile([NP, NI], F32, name="crmax")
    nc.vector.tensor_reduce(out=posmax[:], in_=m2[:], axis=AX.X, op=AL.max)
    nc.vector.tensor_reduce(out=crmax[:], in_=c2[:], axis=AX.X, op=AL.max)

    vprop = ph0.tile([NP, NI], F32, name="vprop")
    nc.vector.tensor_reduce(out=vprop[:], in_=prop[:], axis=AX.X, op=AL.add,
                            apply_absolute_value=True)
    nc.vector.tensor_scalar(out=vprop[:], in0=vprop[:], scalar1=0.0, scalar2=None,
                            op0=AL.is_gt)
    pos_n = glob.tile([NP, NI], F32, name="pos_n")
    nc.vector.tensor_scalar(out=pos_n[:], in0=posmax[:], scalar1=0.0, scalar2=None,
                            op0=AL.is_ge)
    neg_n = glob.tile([NP, NI], F32, name="neg_n")
    nc.vector.tensor_scalar(out=neg_n[:], in0=posmax[:], scalar1=0.0, scalar2=None,
                            op0=AL.is_lt)
    nocr = ph0.tile([NP, NI], F32, name="nocr")
    nc.vector.tensor_scalar(out=nocr[:], in0=crmax[:], scalar1=0.0, scalar2=None,
                            op0=AL.is_lt)
    nc.vector.tensor_tensor(out=neg_n[:], in0=neg_n[:], in1=nocr[:], op=AL.mult)
    nc.vector.tensor_tensor(out=neg_n[:], in0=neg_n[:], in1=vprop[:], op=AL.mult)

    # ---------- phase 0c: ranks + scalars + ind + sel ----------
    psel = stack0.enter_context(tc.tile_pool(name="psel", bufs=1))
    psum0 = stack0.enter_context(tc.tile_pool(name="psum0", bufs=1, space="PSUM"))

    zz = psel.tile([NP, NI], F32, name="zz")
    nc.vector.memset(zz[:], 0.0)
    scp = psel.tile([NP, NI], F32, name="scp")
    nc.vector.tensor_tensor_scan(out=scp[:], data0=pos_n[:], data1=zz[:], initial=0.0,
                                 op0=AL.add, op1=AL.add)
    scn = psel.tile([NP, NI], F32, name="scn")
    nc.vector.tensor_tensor_scan(out=scn[:], data0=neg_n[:], data1=zz[:], initial=0.0,
                                 op0=AL.add, op1=AL.add)
    tot2 = psel.tile([128, 2], F32, name="tot2")
    nc.vector.memset(tot2[:], 0.0)
    nc.vector.tensor_copy(tot2[0:NP, 0:1], scp[:, NI - 1:NI])
    nc.vector.tensor_copy(tot2[0:NP, 1:2], scn[:, NI - 1:NI])
    pref_ps = psum0.tile([128, 2], F32, space="PSUM", name="pref_ps")
    nc.tensor.matmul(pref_ps[:], lhsT=tril[:], rhs=tot2[:], start=True, stop=True)
    gtot_ps = psum0.tile([1, 2], F32, space="PSUM", name="gtot_ps")
    nc.tensor.matmul(gtot_ps[:], lhsT=ones_col[:], rhs=tot2[:], start=True, stop=True)
    pref = psel.tile([128, 2], F32, name="pref")
    nc.vector.tensor_copy(pref[:], pref_ps[:])
    nc.vector.tensor_tensor(out=pref[:], in0=pref[:], in1=tot2[:], op=AL.subtract)  # exclusive
    gtot = psel.tile([1, 2], F32, name="gtot")
    nc.vector.tensor_copy(gtot[:], gtot_ps[:])

    # ranks (0-based among class): rank = scan_incl - v + excl_prefix
    rkp = psel.tile([NP, NI], F32, name="rkp")
    nc.vector.tensor_tensor(out=rkp[:], in0=scp[:], in1=pos_n[:], op=AL.subtract)
    nc.vector.tensor_tensor(out=rkp[:], in0=rkp[:],
                            in1=pref[0:NP, 0:1].to_broadcast([NP, NI]), op=AL.add)
    rkn = psel.tile([NP, NI], F32, name="rkn")
    nc.vector.tensor_tensor(out=rkn[:], in0=scn[:], in1=neg_n[:], op=AL.subtract)
    nc.vector.tensor_tensor(out=rkn[:], in0=rkn[:],
                            in1=pref[0:NP, 1:2].to_broadcast([NP, NI]), op=AL.add)

    # scalars on partition 0
    npos = psel.tile([1, 1], F32, name="npos")
    nc.vector.tensor_scalar(out=npos[:], in0=gtot[:, 0:1], scalar1=float(PM),
                            scalar2=None, op0=AL.min)
    twant = psel.tile([1, 1], F32, name="twant")
    nc.vector.tensor_scalar(out=twant[:], in0=npos[:], scalar1=float(R_RATIO),
                            scalar2=None, op0=AL.mult)
    # floor(twant)
    ti_ = psel.tile([1, 1], I32, name="ti_")
    nc.vector.tensor_copy(ti_[:], twant[:])
    tf_ = psel.tile([1, 1], F32, name="tf_")
    nc.vector.tensor_copy(tf_[:], ti_[:])
    adj_ = psel.tile([1, 1], F32, name="adj_")
    nc.vector.tensor_tensor(out=adj_[:], in0=tf_[:], in1=twant[:], op=AL.is_gt)
    nc.vector.tensor_tensor(out=tf_[:], in0=tf_[:], in1=adj_[:], op=AL.subtract)
    nwant = psel.tile([1, 1], F32, name="nwant")
    nc.vector.tensor_tensor(out=nwant[:], in0=tf_[:], in1=npos[:], op=AL.subtract)
    nneg = psel.tile([1, 1], F32, name="nneg")
    nc.vector.tensor_tensor(out=nneg[:], in0=nwant[:], in1=gtot[:, 1:2], op=AL.min)
    nc.vector.tensor_scalar(out=nneg[:], in0=nneg[:], scalar1=0.0, scalar2=None, op0=AL.max)
    cap = psel.tile([1, 1], F32, name="cap")
    nc.vector.tensor_scalar(out=cap[:], in0=npos[:], scalar1=-1.0, scalar2=float(T),
                            op0=AL.mult, op1=AL.add)
    nc.vector.tensor_tensor(out=nneg[:], in0=nneg[:], in1=cap[:], op=AL.min)
    npn = psel.tile([1, 1], F32, name="npn")
    nc.vector.tensor_tensor(out=npn[:], in0=npos[:], in1=nneg[:], op=AL.add)
    s1000 = psel.tile([1, 1], F32, name="s1000")
    nc.vector.tensor_scalar(out=s1000[:], in0=npos[:], scalar1=-1.0, scalar2=1000.0,
                            op0=AL.mult, op1=AL.add)
    scal = psel.tile([1, 16], F32, name="scal")
    nc.vector.memset(scal[:], 0.0)
    nc.vector.tensor_copy(scal[:, 0:1], npos[:])
    nc.vector.tensor_copy(scal[:, 1:2], nneg[:])
    nc.vector.tensor_copy(scal[:, 2:3], npn[:])
    nc.vector.tensor_copy(scal[:, 3:4], s1000[:])
    nc.sync.dma_start(d_scal[:], scal[:])
    scalb = glob.tile([128, 16], F32, name="scalb")
    nc.sync.dma_start(scalb[:], bass.AP(tensor=d_scal[:].tensor, offset=0,
                                        ap=[[0, 128], [1, 16]]))

    # jc row: j < npos ? j : j + 1000 - npos
    crow = psel.tile([1, T], F32, name="crow")
    nc.vector.tensor_tensor(out=crow[:], in0=jio_f[:],
                            in1=npos[:].to_broadcast([1, T]), op=AL.is_ge)
    jc = psel.tile([1, T], F32, name="jc")
    nc.vector.tensor_tensor(out=jc[:], in0=crow[:],
                            in1=s1000[:].to_broadcast([1, T]), op=AL.mult)
    nc.vector.tensor_tensor(out=jc[:], in0=jc[:], in1=jio_f[:], op=AL.add)
    nc.sync.dma_start(d_jc[:], jc[:])
    jcb = psel.tile([128, T], F32, name="jcb")
    nc.sync.dma_start(jcb[:], bass.AP(tensor=d_jc[:].tensor, offset=0,
                                      ap=[[0, 128], [1, T]]))

    # rc = (rkp + 9)*pos + (rkn + 1009)*neg - 9
    rc = psel.tile([NP, NI], F32, name="rc")
    nc.vector.tensor_scalar(out=rc[:], in0=rkp[:], scalar1=9.0, scalar2=None, op0=AL.add)
    nc.vector.tensor_tensor(out=rc[:], in0=rc[:], in1=pos_n[:], op=AL.mult)
    rc2 = psel.tile([NP, NI], F32, name="rc2")
    nc.vector.tensor_scalar(out=rc2[:], in0=rkn[:], scalar1=1009.0, scalar2=None, op0=AL.add)
    nc.vector.tensor_tensor(out=rc2[:], in0=rc2[:], in1=neg_n[:], op=AL.mult)
    nc.vector.tensor_tensor(out=rc[:], in0=rc[:], in1=rc2[:], op=AL.add)
    nc.vector.tensor_scalar(out=rc[:], in0=rc[:], scalar1=9.0, scalar2=None, op0=AL.subtract)

    # ind [NP, NI, T] = (rc == jc)
    ind = psel.tile([NP, NI, T], F32, name="ind")
    ia = rc[:]
    ia = bass.AP(tensor=ia.tensor, offset=ia.offset, ap=[ia.ap[0], ia.ap[1], [0, T]])
    ib = jcb[0:NP, :]
    ib = bass.AP(tensor=ib.tensor, offset=ib.offset, ap=[ib.ap[0], [0, NI], ib.ap[1]])
    nc.vector.tensor_tensor(out=ind[:], in0=ia, in1=ib, op=AL.is_equal)

    # sel via fp32 matmuls: out [100, 2] per j-half (col h)
    sel_ps = psum0.tile([G, 2], F32, space="PSUM", name="sel_ps")
    for h in (0, 1):
        for i in range(NI):
            nc.tensor.matmul(sel_ps[:, h:h + 1], lhsT=ind[:, i, h * G:(h + 1) * G],
                             rhs=nio_f[:, i:i + 1], start=(i == 0), stop=(i == NI - 1),
                             skip_group_check=True)
    sel_f = glob.tile([G, 2], F32, name="sel_f")
    nc.vector.tensor_copy(sel_f[:], sel_ps[:])
    sel_i = glob.tile([G, 2], I32, name="sel_i")
    nc.vector.tensor_copy(sel_i[:], sel_f[:])
    stack0.close()

    # ---------- phase 0d: gathers + per-roi prep ----------
    # gathers: proposals rows for both halves; aux/inter/union rows for half 0
    prop_g0 = glob.tile([G, 4], F32, name="prop_g0")
    nc.gpsimd.indirect_dma_start(out=prop_g0[:], out_offset=None, in_=t_prop[:],
                                 in_offset=bass.IndirectOffsetOnAxis(ap=sel_i[:, 0:1], axis=0))
    prop_g1 = glob.tile([G, 4], F32, name="prop_g1")
    nc.gpsimd.indirect_dma_start(out=prop_g1[:], out_offset=None, in_=t_prop[:],
                                 in_offset=bass.IndirectOffsetOnAxis(ap=sel_i[:, 1:2], axis=0))
    aux_g = glob.tile([G, 4], F32, name="aux_g")
    nc.gpsimd.indirect_dma_start(out=aux_g[:], out_offset=None, in_=t_aux[:],
                                 in_offset=bass.IndirectOffsetOnAxis(ap=sel_i[:, 0:1], axis=0))
    int_r = glob.tile([G, G], F32, name="int_r")
    nc.gpsimd.indirect_dma_start(out=int_r[:], out_offset=None, in_=d_int[:],
                                 in_offset=bass.IndirectOffsetOnAxis(ap=sel_i[:, 0:1], axis=0))
    uni_r = glob.tile([G, G], F32, name="uni_r")
    nc.gpsimd.indirect_dma_start(out=uni_r[:], out_offset=None, in_=d_uni[:],
                                 in_offset=bass.IndirectOffsetOnAxis(ap=sel_i[:, 0:1], axis=0))

    # iou over selected rows + argmax over fg
    usafe = glob.tile([G, G], F32, name="usafe")
    nc.vector.tensor_scalar(out=usafe[:], in0=uni_r[:], scalar1=0.0, scalar2=None, op0=AL.is_le)
    nc.vector.tensor_tensor(out=usafe[:], in0=usafe[:], in1=uni_r[:], op=AL.add)
    nc.vector.reciprocal(usafe[:], usafe[:])
    iou_s = glob.tile([G, G], F32, name="iou_s")
    nc.vector.tensor_tensor(out=iou_s[:], in0=int_r[:], in1=usafe[:], op=AL.mult)
    # where(fg, iou, -1) = (iou + 1) * fg - 1
    nc.vector.tensor_scalar(out=iou_s[:], in0=iou_s[:], scalar1=1.0, scalar2=None, op0=AL.add)
    nc.vector.tensor_tensor(out=iou_s[:], in0=iou_s[:],
                            in1=packb[0:G, 5, :], op=AL.mult)
    nc.vector.tensor_scalar(out=iou_s[:], in0=iou_s[:], scalar1=1.0, scalar2=None,
                            op0=AL.subtract)
    rmax = glob.tile([G, 1], F32, name="rmax")
    nc.vector.tensor_reduce(out=rmax[:], in_=iou_s[:], axis=AX.X, op=AL.max)
    eqm = glob.tile([G, G], F32, name="eqm")
    nc.vector.tensor_tensor(out=eqm[:], in0=iou_s[:], in1=rmax[:].to_broadcast([G, G]),
                            op=AL.is_equal)
    # first-index argmax: max over (99 - g) of eq -> assign = 99 - that
    revg = glob.tile([G, G], F32, name="revg")
    nc.vector.tensor_scalar(out=revg[:], in0=jio_all_f[0:G, 0:G], scalar1=-1.0,
                            scalar2=99.0, op0=AL.mult, op1=AL.add)
    nc.vector.tensor_tensor(out=eqm[:], in0=eqm[:], in1=revg[:], op=AL.mult)
    asg = glob.tile([G, 1], F32, name="asg")
    nc.vector.tensor_reduce(out=asg[:], in_=eqm[:], axis=AX.X, op=AL.max)
    nc.vector.tensor_scalar(out=asg[:], in0=asg[:], scalar1=-1.0, scalar2=99.0,
                            op0=AL.mult, op1=AL.add)
    asg_i = glob.tile([G, 1], I32, name="asg_i")
    nc.vector.tensor_copy(asg_i[:], asg[:])
    # assign as a free-major row for per-roi register loads
    nc.sync.dma_start(d_asg[:].rearrange("a g -> g a"), asg[:])
    asg_row = glob.tile([1, G], F32, name="asg_row")
    nc.sync.dma_start(asg_row[:], d_asg[:])
    asg_row_i = glob.tile([1, G], I32, name="asg_row_i")
    nc.vector.tensor_copy(asg_row_i[:], asg_row[:])

    # gt gathers by assign (half 0 only)
    gtb_g = glob.tile([G, 4], F32, name="gtb_g")
    nc.gpsimd.indirect_dma_start(out=gtb_g[:], out_offset=None, in_=t_gtb[:],
                                 in_offset=bass.IndirectOffsetOnAxis(ap=asg_i[:, 0:1], axis=0))
    cls_g = glob.tile([G, 1], I32, name="cls_g")
    nc.gpsimd.indirect_dma_start(out=cls_g[:], out_offset=None, in_=t_gtc[:],
                                 in_offset=bass.IndirectOffsetOnAxis(ap=asg_i[:, 0:1], axis=0))

    # per-j masks/flags
    isp_c = glob.tile([G, 1], F32, name="isp_c")
    nc.vector.tensor_tensor(out=isp_c[:], in0=pio_f[0:G, 0:1],
                            in1=scalb[0:G, 0:1], op=AL.is_lt)
    keep0 = glob.tile([G, 1], F32, name="keep0")
    nc.vector.tensor_tensor(out=keep0[:], in0=pio_f[0:G, 0:1],
                            in1=scalb[0:G, 2:3], op=AL.is_lt)
    keep1 = glob.tile([G, 1], F32, name="keep1")
    nc.vector.tensor_tensor(out=keep1[:], in0=pio_f[0:G, 1:2],
                            in1=scalb[0:G, 2:3], op=AL.is_lt)

    # rois out
    rois0 = glob.tile([G, 4], F32, name="rois0")
    nc.vector.tensor_tensor(out=rois0[:], in0=prop_g0[:],
                            in1=keep0[:].to_broadcast([G, 4]), op=AL.mult)
    nc.sync.dma_start(o_rois[0:G, :], rois0[:])
    rois1 = glob.tile([G, 4], F32, name="rois1")
    nc.vector.tensor_tensor(out=rois1[:], in0=prop_g1[:],
                            in1=keep1[:].to_broadcast([G, 4]), op=AL.mult)
    nc.sync.dma_start(o_rois[G:T, :], rois1[:])

    # class out
    clsf_g = glob.tile([G, 1], F32, name="clsf_g")
    nc.vector.tensor_copy(clsf_g[:], cls_g[:])
    nc.vector.tensor_tensor(out=clsf_g[:], in0=clsf_g[:], in1=isp_c[:], op=AL.mult)
    cls_o = glob.tile([G, 1], I32, name="cls_o")
    nc.vector.tensor_copy(cls_o[:], clsf_g[:])
    nc.sync.dma_start(o_cls[0:G, :], cls_o[:])
    zero_i = glob.tile([G, 1], I32, name="zero_i")
    nc.gpsimd.memset(zero_i[:], 0)
    nc.sync.dma_start(o_cls[G:T, :], zero_i[:])

    # p_roi / p_gt with dummy [0,0,1,1]
    p_roi = glob.tile([G, 4], F32, name="p_roi")
    nc.vector.tensor_tensor(out=p_roi[:], in0=rois0[:],
                            in1=isp_c[:].to_broadcast([G, 4]), op=AL.mult)
    ispm1 = glob.tile([G, 1], F32, name="ispm1")
    nc.vector.tensor_scalar(out=ispm1[:], in0=isp_c[:], scalar1=-1.0, scalar2=1.0,
                            op0=AL.mult, op1=AL.add)
    nc.vector.tensor_tensor(out=p_roi[:, 2:4], in0=p_roi[:, 2:4],
                            in1=ispm1[:].to_broadcast([G, 2]), op=AL.add)
    p_gt = glob.tile([G, 4], F32, name="p_gt")
    nc.vector.tensor_tensor(out=p_gt[:], in0=gtb_g[:],
                            in1=isp_c[:].to_broadcast([G, 4]), op=AL.mult)
    nc.vector.tensor_tensor(out=p_gt[:, 2:4], in0=p_gt[:, 2:4],
                            in1=ispm1[:].to_broadcast([G, 2]), op=AL.add)

    # deltas
    def hwcxy(src, nm):
        h = glob.tile([G, 1], F32, name=nm + "h")
        nc.vector.tensor_tensor(out=h[:], in0=src[:, 2:3], in1=src[:, 0:1], op=AL.subtract)
        w = glob.tile([G, 1], F32, name=nm + "w")
        nc.vector.tensor_tensor(out=w[:], in0=src[:, 3:4], in1=src[:, 1:2], op=AL.subtract)
        cy = glob.tile([G, 1], F32, name=nm + "cy")
        nc.vector.tensor_scalar(out=cy[:], in0=h[:], scalar1=0.5, scalar2=None, op0=AL.mult)
        nc.vector.tensor_tensor(out=cy[:], in0=cy[:], in1=src[:, 0:1], op=AL.add)
        cx = glob.tile([G, 1], F32, name=nm + "cx")
        nc.vector.tensor_scalar(out=cx[:], in0=w[:], scalar1=0.5, scalar2=None, op0=AL.mult)
        nc.vector.tensor_tensor(out=cx[:], in0=cx[:], in1=src[:, 1:2], op=AL.add)
        return h, w, cy, cx

    rh_, rw_, rcy, rcx = hwcxy(p_roi, "r")
    gh2, gw2, gcy, gcx = hwcxy(p_gt, "g")
    irh = glob.tile([G, 1], F32, name="irh")
    nc.vector.reciprocal(irh[:], rh_[:])
    irw = glob.tile([G, 1], F32, name="irw")
    nc.vector.reciprocal(irw[:], rw_[:])
    dlt = glob.tile([G, 4], F32, name="dlt")
    nc.vector.tensor_tensor(out=dlt[:, 0:1], in0=gcy[:], in1=rcy[:], op=AL.subtract)
    nc.vector.tensor_tensor(out=dlt[:, 0:1], in0=dlt[:, 0:1], in1=irh[:], op=AL.mult)
    nc.vector.tensor_tensor(out=dlt[:, 1:2], in0=gcx[:], in1=rcx[:], op=AL.subtract)
    nc.vector.tensor_tensor(out=dlt[:, 1:2], in0=dlt[:, 1:2], in1=irw[:], op=AL.mult)
    rat = glob.tile([G, 2], F32, name="rat")
    nc.vector.tensor_tensor(out=rat[:, 0:1], in0=gh2[:], in1=irh[:], op=AL.mult)
    nc.vector.tensor_tensor(out=rat[:, 1:2], in0=gw2[:], in1=irw[:], op=AL.mult)
    nc.scalar.activation(dlt[:, 2:3], rat[:, 0:1], mybir.ActivationFunctionType.Ln)
    nc.scalar.activation(dlt[:, 3:4], rat[:, 1:2], mybir.ActivationFunctionType.Ln)
    nc.vector.tensor_scalar(out=dlt[:, 0:2], in0=dlt[:, 0:2], scalar1=10.0, scalar2=None,
                            op0=AL.mult)
    nc.vector.tensor_scalar(out=dlt[:, 2:4], in0=dlt[:, 2:4], scalar1=5.0, scalar2=None,
                            op0=AL.mult)
    nc.vector.tensor_tensor(out=dlt[:], in0=dlt[:], in1=isp_c[:].to_broadcast([G, 4]),
                            op=AL.mult)
    nc.sync.dma_start(o_dlt[0:G, :], dlt[:])
    zero_d = glob.tile([G, 4], F32, name="zero_d")
    nc.vector.memset(zero_d[:], 0.0)
    nc.sync.dma_start(o_dlt[G:T, :], zero_d[:])

    # ---------- mask grid prep ----------
    # ys = s*by + ay ; xs = s*bx + ax   (aux cols: ay 0, by 1, ax 2, bx 3)
    def grid(acol, bcol, d01, dwT_or_wx, transposed_w):
        yst = glob.tile([G, S], F32, name="yst" + str(acol))
        nc.vector.tensor_scalar(out=yst[:], in0=sio_f[:], scalar1=aux_g[:, bcol:bcol + 1],
                                scalar2=aux_g[:, acol:acol + 1], op0=AL.mult, op1=AL.add)
        yi0 = glob.tile([G, S], I32, name="yi0" + str(acol))
        nc.vector.tensor_copy(yi0[:], yst[:])
        y0f = glob.tile([G, S], F32, name="y0f" + str(acol))
        nc.vector.tensor_copy(y0f[:], yi0[:])
        adj = glob.tile([G, S], F32, name="adj" + str(acol))
        nc.vector.tensor_tensor(out=adj[:], in0=y0f[:], in1=yst[:], op=AL.is_gt)
        nc.vector.tensor_tensor(out=y0f[:], in0=y0f[:], in1=adj[:], op=AL.subtract)
        wy = glob.tile([G, S], F32, name="wy" + str(acol))
        nc.vector.tensor_tensor(out=wy[:], in0=yst[:], in1=y0f[:], op=AL.subtract)
        oky = glob.tile([G, S], F32, name="oky" + str(acol))
        nc.vector.tensor_scalar(out=oky[:], in0=yst[:], scalar1=0.0, scalar2=None, op0=AL.is_ge)
        ok2 = glob.tile([G, S], F32, name="ok2" + str(acol))
        nc.vector.tensor_scalar(out=ok2[:], in0=yst[:], scalar1=511.0, scalar2=None, op0=AL.is_le)
        nc.vector.tensor_tensor(out=oky[:], in0=oky[:], in1=ok2[:], op=AL.mult)
        y01 = glob.tile([G, 64], F32, name="y01_" + str(acol))
        nc.vector.memset(y01[:], -1.0)
        # y0i' = (clip(y0) + 1) * oky - 1 ; y1i' = (clip(y0+1) + 1) * oky - 1
        c0 = glob.tile([G, S], F32, name="c0" + str(acol))
        nc.vector.tensor_scalar(out=c0[:], in0=y0f[:], scalar1=0.0, scalar2=511.0,
                                op0=AL.max, op1=AL.min)
        nc.vector.tensor_scalar(out=c0[:], in0=c0[:], scalar1=1.0, scalar2=None, op0=AL.add)
        nc.vector.tensor_tensor(out=c0[:], in0=c0[:], in1=oky[:], op=AL.mult)
        nc.vector.tensor_scalar(out=y01[:, 0:S], in0=c0[:], scalar1=1.0, scalar2=None,
                                op0=AL.subtract)
        c1 = glob.tile([G, S], F32, name="c1" + str(acol))
        nc.vector.tensor_scalar(out=c1[:], in0=y0f[:], scalar1=1.0, scalar2=511.0,
                                op0=AL.add, op1=AL.min)
        nc.vector.tensor_scalar(out=c1[:], in0=c1[:], scalar1=0.0, scalar2=1.0,
                                op0=AL.max, op1=AL.add)
        nc.vector.tensor_tensor(out=c1[:], in0=c1[:], in1=oky[:], op=AL.mult)
        nc.vector.tensor_scalar(out=y01[:, 32:32 + S], in0=c1[:], scalar1=1.0, scalar2=None,
                                op0=AL.subtract)
        y01_16 = glob.tile([G, 64], F16, name="y01_16_" + str(acol))
        nc.vector.tensor_copy(y01_16[:], y01[:])
        nc.sync.dma_start(d01[:], y01_16[0:PM, :])
        if transposed_w:
            nc.sync.dma_start(dwT_or_wx[:].rearrange("s j -> j s"), wy[0:PM, :])
        else:
            nc.sync.dma_start(dwT_or_wx[:], wy[0:PM, :])

    grid(0, 1, d_y01, d_wyT, True)    # y
    grid(2, 3, d_x01, d_wx, False)    # x

    # broadcast reads
    y01b = glob.tile([128, PM, 64], F16, name="y01b")
    nc.sync.dma_start(y01b[:], bass.AP(tensor=d_y01[:].tensor, offset=0,
                                       ap=[[0, 128], [64, PM], [1, 64]]))
    x01b = glob.tile([128, PM, 64], F16, name="x01b")
    nc.sync.dma_start(x01b[:], bass.AP(tensor=d_x01[:].tensor, offset=0,
                                       ap=[[0, 128], [64, PM], [1, 64]]))
    wyT = glob.tile([S, PM], F32, name="wyT")
    nc.sync.dma_start(wyT[:], d_wyT[:])
    wxb = glob.tile([S, PM, S], F32, name="wxb")
    nc.sync.dma_start(wxb[:], bass.AP(tensor=d_wx[:].tensor, offset=0,
                                      ap=[[0, S], [S, PM], [1, S]]))
    wyT1m = glob.tile([S, PM], F32, name="wyT1m")
    nc.vector.tensor_scalar(out=wyT1m[:], in0=wyT[:], scalar1=-1.0, scalar2=1.0,
                            op0=AL.mult, op1=AL.add)
    wxb1m = glob.tile([S, PM, S], F32, name="wxb1m")
    nc.vector.tensor_scalar(out=wxb1m[:], in0=wxb[:], scalar1=-1.0, scalar2=1.0,
                            op0=AL.mult, op1=AL.add)

    # is_p as free-dim row on all partitions [128, T]
    ispb = glob.tile([128, T], F32, name="ispb")
    nc.vector.tensor_tensor(out=ispb[:], in0=jio_all_f[:],
                            in1=scalb[:, 0:1].to_broadcast([128, T]), op=AL.is_lt)

    # ---------- registers: jlo on the engines that need it + g regs ----------
    jvs = {}
    for eng, nm in ((nc.vector, "dve"), (nc.scalar, "act"), (nc.gpsimd, "pool"),
                    (nc.sync, "sp")):
        r = eng.alloc_register("jlo_" + nm)
        eng.reg_load(r, meta[0:1, 0:1])
        jvs[nm] = eng.snap(r, donate=True, min_val=0, max_val=PM - NJ)

    ext_engs = [(nc.scalar, "act"), (nc.vector, "dve"), (nc.scalar, "act")]
    gregs = []
    for j in range(NJ):
        eng, nm = ext_engs[j % 3]
        r = eng.alloc_register(f"g_{j}")
        eng.reg_load(r, asg_row_i[0:1, ds(jvs[nm] + j, 1)])
        gregs.append(eng.snap(r, donate=True, min_val=0, max_val=G - 1))

    # Sx [128, 4, NJ, 56] bf16 (4 ops)
    sxb = glob.tile([128, 4, NJ, 64], BF16, name="sxb")
    x01s = x01b[:, ds(jvs["dve"], NJ), :]
    for xc in range(4):
        nc.vector.tensor_tensor(
            out=sxb[:, xc, :, :].rearrange("p a b -> p (a b)"),
            in0=cio_16[:, xc:xc + 1].to_broadcast([128, NJ * 64]),
            in1=x01s.rearrange("p a b -> p (a b)"), op=AL.is_equal)

    # ---------- slab sweep ----------
    slabp = stack.enter_context(tc.tile_pool(name="slabp", bufs=4))
    mskp = stack.enter_context(tc.tile_pool(name="mskp", bufs=2))
    psum_m1 = stack.enter_context(tc.tile_pool(name="psum_m1", bufs=2, space="PSUM"))
    psum_o2 = stack.enter_context(tc.tile_pool(name="psum_o2", bufs=1, space="PSUM"))

    o2a = psum_o2.tile([64, 8 * 64], F32, space="PSUM", name="o2a")
    o2b = psum_o2.tile([64, 8 * 64], F32, space="PSUM", name="o2b")
    o2c = psum_o2.tile([64, 64], F32, space="PSUM", name="o2c")
    # one start=True zeroing matmul per tile: later MMs accumulate (start=False).
    zcol = glob.tile([1, 64], BF16, name="zcol")
    nc.vector.memset(zcol[:], 0.0)
    zrow = glob.tile([1, 512], BF16, name="zrow")
    nc.vector.memset(zrow[:], 0.0)
    nc.tensor.matmul(o2a[:], lhsT=zcol[:], rhs=zrow[:], start=True, stop=False,
                     skip_group_check=True)
    nc.tensor.matmul(o2b[:], lhsT=zcol[:], rhs=zrow[:], start=True, stop=False,
                     skip_group_check=True)
    nc.tensor.matmul(o2c[:], lhsT=zcol[:], rhs=zrow[:, 0:64], start=True, stop=False,
                     skip_group_check=True)

    def o2slice(j):
        if j < 8:
            return o2a[:, j * 64:(j + 1) * 64]
        if j < 16:
            return o2b[:, (j - 8) * 64:(j - 7) * 64]
        return o2c[:]

    msk3 = t_msk[:].rearrange("r (x g) -> r x g", g=G)
    for sl in range(4):
        halves = []
        for hf in range(2):
            sh = slabp.tile([128, 256, G], U8, name=f"sh_{sl}_{hf}", tag="slab")
            nc.sync.dma_start(sh[:], msk3[sl * 128:(sl + 1) * 128,
                                          hf * 256:(hf + 1) * 256, :])
            halves.append(sh)
        # Sy for this slab [128, NJ, 56]
        syt = mskp.tile([128, NJ, 64], BF16, name=f"sy_{sl}", tag="sy")
        y01s = y01b[:, ds(jvs["dve"], NJ), :]
        nc.vector.tensor_tensor(
            out=syt[:].rearrange("p a b -> p (a b)"),
            in0=cio_16[:, sl:sl + 1].to_broadcast([128, NJ * 64]),
            in1=y01s.rearrange("p a b -> p (a b)"), op=AL.is_equal)
        for grp in range(5):
            js = list(range(grp * 4, min(grp * 4 + 4, NJ)))
            m1ps_a = psum_m1.tile([128, 8 * 64], F32, space="PSUM",
                                  name=f"m1psa_{sl}_{grp}", tag="m1psa")
            m1ps_b = (psum_m1.tile([128, 8 * 64], F32, space="PSUM",
                                   name=f"m1psb_{sl}_{grp}", tag="m1psb")
                      if len(js) * 4 > 8 else None)

            def m1slice(idx):
                t = m1ps_a if idx < 8 else m1ps_b
                ii = idx if idx < 8 else idx - 8
                return t[:, ii * 64:(ii + 1) * 64]
            Es = []
            for k, j in enumerate(js):
                eng, nm = ext_engs[j % 3]
                E = mskp.tile([128, 512], BF16, name=f"E_{sl}_{j}", tag="E")
                gj = gregs[j]
                for hf in range(2):
                    src = halves[hf][:, :, ds(gj, 1)]
                    dst = E[:, hf * 256:(hf + 1) * 256]
                    if nm == "act":
                        nc.scalar.copy(dst, src)
                    else:
                        eng.tensor_copy(dst, src)
                Es.append(E)
                for xc in range(4):
                    nc.tensor.matmul(
                        m1slice(k * 4 + xc),
                        lhsT=E[:, xc * 128:(xc + 1) * 128],
                        rhs=syt[:, j, :], start=True, stop=True,
                        skip_group_check=True)
            m1sb = mskp.tile([128, 16 * 64], BF16, name=f"m1sb_{sl}_{grp}", tag="m1sb")
            used = len(js) * 4
            nc.scalar.copy(m1sb[:, 0:min(used, 8) * 64], m1ps_a[:, 0:min(used, 8) * 64])
            if used > 8:
                nc.scalar.copy(m1sb[:, 512:512 + (used - 8) * 64],
                               m1ps_b[:, 0:(used - 8) * 64])
            for k, j in enumerate(js):
                for xc in range(4):
                    nc.tensor.matmul(
                        o2slice(j),
                        lhsT=m1sb[:, (k * 4 + xc) * 64:(k * 4 + xc + 1) * 64],
                        rhs=sxb[:, xc, j, :],
                        start=False, stop=(sl == 3 and xc == 3),
                        skip_group_check=True)

    # ---------- bilinear + threshold + masks out ----------
    val = glob.tile([64, NJ, 64], F32, name="val")
    nc.vector.tensor_copy(val[:, 0:8, :].rearrange("p a b -> p (a b)"), o2a[:])
    nc.vector.tensor_copy(val[:, 8:16, :].rearrange("p a b -> p (a b)"), o2b[:])
    nc.vector.tensor_copy(val[:, 16:NJ, :].rearrange("p a b -> p (a b)"), o2c[:])

    wx_s = wxb[:, ds(jvs["dve"], NJ), :]
    wx1m_s = wxb1m[:, ds(jvs["dve"], NJ), :]
    v00 = val[0:S, :, 0:S]
    v01 = val[0:S, :, 32:32 + S]
    v10 = val[32:32 + S, :, 0:S]
    v11 = val[32:32 + S, :, 32:32 + S]
    pa = glob.tile([S, NJ, S], F32, name="pa")
    pb = glob.tile([S, NJ, S], F32, name="pb")
    nc.vector.tensor_tensor(out=pa[:], in0=v00, in1=wx1m_s, op=AL.mult)
    nc.vector.tensor_tensor(out=pb[:], in0=v01, in1=wx_s, op=AL.mult)
    nc.vector.tensor_tensor(out=pa[:], in0=pa[:], in1=pb[:], op=AL.add)
    pc_ = glob.tile([S, NJ, S], F32, name="pc_")
    pd_ = glob.tile([S, NJ, S], F32, name="pd_")
    nc.vector.tensor_tensor(out=pc_[:], in0=v10, in1=wx1m_s, op=AL.mult)
    nc.vector.tensor_tensor(out=pd_[:], in0=v11, in1=wx_s, op=AL.mult)
    nc.vector.tensor_tensor(out=pc_[:], in0=pc_[:], in1=pd_[:], op=AL.add)
    wy_s = wyT[:, ds(jvs["dve"], NJ)]
    wy1m_s = wyT1m[:, ds(jvs["dve"], NJ)]

    def wcast(a):
        return bass.AP(tensor=a.tensor, offset=a.offset, ap=[a.ap[0], a.ap[1], [0, S]])

    nc.vector.tensor_tensor(out=pa[:], in0=pa[:], in1=wcast(wy1m_s), op=AL.mult)
    nc.vector.tensor_tensor(out=pc_[:], in0=pc_[:], in1=wcast(wy_s), op=AL.mult)
    nc.vector.tensor_tensor(out=pa[:], in0=pa[:], in1=pc_[:], op=AL.add)
    nc.vector.tensor_scalar(out=pa[:], in0=pa[:], scalar1=0.5, scalar2=None, op0=AL.is_gt)
    isp_s = ispb[0:S, ds(jvs["dve"], NJ)]
    nc.vector.tensor_tensor(out=pa[:], in0=pa[:], in1=wcast(isp_s), op=AL.mult)
    nc.sync.dma_start(o_msk[:, ds(jvs["sp"], NJ), :], pa[:])

    stack.close()


_PROG = None


def _get_prog():
    global _PROG
    if _PROG is None:
        _PROG = build_program()
    return _PROG


def _prep_inmaps(proposals, gt_class_ids, gt_boxes, gt_masks):
    proposals = np.asarray(proposals, dtype=np.float32)
    gt_class_ids = np.asarray(gt_class_ids, dtype=np.int32)
    gt_boxes = np.asarray(gt_boxes, dtype=np.float32)
    gt_masks_u8 = np.asarray(gt_masks).astype(np.uint8)

    # host-side exact grid coefficients (f32, same op order as reference)
    c511 = np.float32(511.0)
    c27 = np.float32(27.0)
    aux = np.empty((B, N, 4), np.float32)
    y1 = proposals[:, :, 0]; x1 = proposals[:, :, 1]
    y2 = proposals[:, :, 2]; x2 = proposals[:, :, 3]
    aux[:, :, 0] = y1 * c511
    aux[:, :, 1] = ((y2 - y1) * c511) / c27
    aux[:, :, 2] = x1 * c511
    aux[:, :, 3] = ((x2 - x1) * c511) / c27

    in_maps = []
    for r in range(8):
        b, rr = r // 4, r % 4
        meta = np.zeros((1, 8), np.int32)
        meta[0, 0] = JLO[rr]
        in_maps.append({
            "prop": proposals[b],
            "aux": aux[b],
            "gtb": gt_boxes[b],
            "gtc": gt_class_ids[b].reshape(G, 1),
            "mskin": gt_masks_u8[b].reshape(HH, HH * G),
            "meta": meta,
        })
    return in_maps


def _assemble(res):
    rois = np.zeros((B, T, 4), np.float32)
    cls = np.zeros((B, T), np.int32)
    dlt = np.zeros((B, T, 4), np.float32)
    msk = np.zeros((B, T, S, S), np.float32)
    for b in range(B):
        r0 = res.results[b * 4]
        rois[b] = r0["o_rois"]
        cls[b] = r0["o_cls"].reshape(T)
        dlt[b] = r0["o_dlt"]
        for rr in range(4):
            jl = JLO[rr]
            jh = JLO[rr + 1] if rr < 3 else PM
            m = res.results[b * 4 + rr]["o_msk"]  # [S, T, S]
            msk[b, jl:jh] = np.transpose(m[:, jl:jh, :], (1, 0, 2))
    return rois, cls, dlt, msk


def kernel(proposals, gt_class_ids, gt_boxes, gt_masks):
    in_maps = _prep_inmaps(proposals, gt_class_ids, gt_boxes, gt_masks)
    nc = _get_prog()
    res = run_bass_kernel_spmd(nc, in_maps, core_ids=list(range(8)))
    return _assemble(res)


def timed_run(proposals, gt_class_ids, gt_boxes, gt_masks):
    """Modeled per-core kernel time. The container has no NTFF profiling hook
    (antenv.axon_hooks absent), so use the production instruction cost model
    (TimelineSim) on the compiled program; all 8 cores run the identical
    program so the max-over-cores exec time equals the single-core model."""
    from concourse.timeline_sim import TimelineSim
    nc = _get_prog()
    t = TimelineSim(nc, no_exec=True, require_finite=False,
                    require_nnan=False).simulate()
    return int(t)
